# revision 1
# baseline (speedup 1.0000x reference)
"""Trainium2 Bass kernel: MultiHeadAttention over [2, 512, 64, 64] images.

Sharding: 8 cores = (2 batches) x (4 head-pairs). Each core computes 2 of the
8 attention heads for one batch plus a partial output projection over its 128
input channels; the host sums the 4 partial projections per batch (the unshard
step for a contraction-dim tensor-parallel split).

Per-core pipeline (all L=4096 positions):
  QKV:  Q/K in [c=128, l] layout (2 heads x 64 dk-channels on partitions),
        V transposed on the PE into VT [s, c] with an interleaved ones column.
  Attn: S^T tiles [s=128, t=512] via row-packed K=64 matmuls (2 heads share
        the PE array), Exp on the scalar engine straight out of PSUM (scores
        are O(+-1.4) so softmax needs no max subtraction), AV matmuls with a
        65th ones-row so the softmax denominator accumulates in PSUM row 64,
        normalization via DVE reciprocal + partition broadcast.
  Proj: partial Wp projection of the normalized heads, bias on one core/batch.

Matmul operands use float32r (~1e-4 rel err, 4x the fp32 matmul rate).
"""

import math
import numpy as np

B, C, HH, WW = 2, 512, 64, 64
L = HH * WW          # 4096
NH, DK = 8, 64
SCALE = 1.0 / math.sqrt(DK)
NCORES = 8

TT = 512             # t-tile width (columns per attention tile)
NT = L // TT         # 8 t-tiles
NS = L // 128        # 32 s-tiles
KT = C // 128        # 4 contraction tiles for projections

_BUILT = {}


def _build(l=L):
    import concourse.bacc as bacc
    import concourse.tile as tile
    import concourse.mybir as mybir
    import concourse.bass as bass
    from concourse.masks import make_identity
    from contextlib import ExitStack

    nt = l // TT
    ns = l // 128
    f32 = mybir.dt.float32
    f16 = mybir.dt.float16
    f32r = mybir.dt.float32r
    Exp = mybir.ActivationFunctionType.Exp
    add = mybir.AluOpType.add

    nc = bacc.Bacc("TRN2", target_bir_lowering=False, debug=False,
                   num_devices=NCORES)

    x = nc.dram_tensor("x", [C, l], f16, kind="ExternalInput").ap()
    wq = nc.dram_tensor("wq", [C, 128], f16, kind="ExternalInput").ap()
    wk = nc.dram_tensor("wk", [C, 128], f16, kind="ExternalInput").ap()
    wv = nc.dram_tensor("wv", [C, 128], f16, kind="ExternalInput").ap()
    bq = nc.dram_tensor("bq", [128, 1], f32, kind="ExternalInput").ap()
    bk = nc.dram_tensor("bk", [128, 1], f32, kind="ExternalInput").ap()
    bv = nc.dram_tensor("bv", [128, 1], f32, kind="ExternalInput").ap()
    wp = nc.dram_tensor("wp", [128, C], f16, kind="ExternalInput").ap()
    bp = nc.dram_tensor("bp", [128, KT], f32, kind="ExternalInput").ap()
    out = nc.dram_tensor("out", [C, l], f32, kind="ExternalOutput").ap()

    with tile.TileContext(nc) as tc, ExitStack() as ctx:
        persist = ctx.enter_context(tc.tile_pool(name="persist", bufs=1))
        e_pool = ctx.enter_context(tc.tile_pool(name="e", bufs=6))
        o_pool = ctx.enter_context(tc.tile_pool(name="o", bufs=2))
        z_pool = ctx.enter_context(tc.tile_pool(name="z", bufs=2))
        res_pool = ctx.enter_context(tc.tile_pool(name="res", bufs=3))

        # ---- weights: fp16 straight from DRAM ----
        w_r = persist.tile([128, 3, KT, 128], f16, tag="wr")
        for i, w in enumerate((wq, wk, wv)):
            for kt in range(KT):
                nc.sync.dma_start(out=w_r[:, i, kt, :],
                                  in_=w[kt * 128:(kt + 1) * 128, :])
        wp_r = persist.tile([128, C], f16, tag="wpr")
        nc.sync.dma_start(out=wp_r, in_=wp)

        bias_sb = persist.tile([128, 3], f32, tag="bias")
        for i, bvec in enumerate((bq, bk, bv)):
            nc.sync.dma_start(out=bias_sb[:, i:i + 1], in_=bvec)
        bp_sb = persist.tile([128, KT], f32, tag="bp")
        nc.sync.dma_start(out=bp_sb, in_=bp)

        ident = persist.tile([128, 128], f16, tag="ident")
        make_identity(nc, ident)

        ones_sb = persist.tile([1, 64], f32, tag="ones")
        nc.vector.memset(ones_sb, 1.0)
        ones_r = persist.tile([1, 64], f32r, tag="onesr")
        nc.vector.tensor_copy(ones_r, ones_sb)

        # ---- persistent activations ----
        q_sb = persist.tile([128, l], f16, tag="q")
        k_sb = persist.tile([128, l], f16, tag="k")
        vt_sb = persist.tile([128, ns, 130], f32r, tag="vt")
        # interleaved ones columns -> softmax denominator rows in AV psum
        ones_col = persist.tile([128, ns, 1], f32, tag="onescol")
        nc.vector.memset(ones_col, 1.0)
        nc.vector.tensor_copy(vt_sb[:, :, 64:65], ones_col)
        nc.vector.tensor_copy(vt_sb[:, :, 129:130], ones_col)

        # ================= QKV projections + V transpose =================
        with ExitStack() as qctx:
            x_pool = qctx.enter_context(tc.tile_pool(name="xs", bufs=3))
            qkv_ps = qctx.enter_context(
                tc.tile_pool(name="qkvps", bufs=2, space="PSUM"))
            v_pool = qctx.enter_context(tc.tile_pool(name="vsb", bufs=2))

            for n in range(nt):
                nsl = slice(n * TT, (n + 1) * TT)
                x_r = x_pool.tile([128, KT, TT], f16, tag="x")
                for kt in range(KT):
                    nc.sync.dma_start(out=x_r[:, kt, :],
                                      in_=x[kt * 128:(kt + 1) * 128, nsl])

                q_ps = qkv_ps.tile([128, TT], f32, tag="qps")
                k_ps = qkv_ps.tile([128, TT], f32, tag="kps")
                v_ps = qkv_ps.tile([128, TT], f32, tag="vps")
                for wi, ps in enumerate((q_ps, k_ps, v_ps)):
                    for kt in range(KT):
                        nc.tensor.matmul(ps, w_r[:, wi, kt, :], x_r[:, kt, :],
                                         start=(kt == 0), stop=(kt == KT - 1))

                nc.vector.tensor_scalar(q_sb[:, nsl], q_ps,
                                        bias_sb[:, 0:1], None, add)
                nc.vector.tensor_scalar(k_sb[:, nsl], k_ps,
                                        bias_sb[:, 1:2], None, add)
                v_sb = v_pool.tile([128, TT], f16, tag="v")
                nc.vector.tensor_scalar(v_sb, v_ps, bias_sb[:, 2:3], None, add)

                # transpose V tile: 4 PE transposes -> [s, c] in psum
                tp = qkv_ps.tile([128, TT], f16, tag="tp")
                for j in range(4):
                    nc.tensor.transpose(tp[:, j * 128:(j + 1) * 128],
                                        v_sb[:, j * 128:(j + 1) * 128], ident)
                tp_v = tp.rearrange("p (j c) -> p j c", j=4)
                ssl = slice(4 * n, 4 * n + 4)
                nc.vector.tensor_copy(vt_sb[:, ssl, 0:64], tp_v[:, :, 0:64])
                nc.vector.tensor_copy(vt_sb[:, ssl, 65:129], tp_v[:, :, 64:128])

        # ========================= attention =========================
        with ExitStack() as actx:
            st_pool = actx.enter_context(
                tc.tile_pool(name="stps", bufs=2, space="PSUM"))
            av_pool = actx.enter_context(
                tc.tile_pool(name="avps", bufs=3, space="PSUM"))
            pr_pool = actx.enter_context(
                tc.tile_pool(name="prps", bufs=1, space="PSUM"))

            def mk_proj(o_sb_t, tsl_t, ot):
                def f():
                    pp = pr_pool.tile([128, TT], f32, tag="pp")
                    nc.tensor.matmul(pp, wp_r[:, ot * 128:(ot + 1) * 128],
                                     o_sb_t, start=True, stop=True)
                    res = res_pool.tile([128, TT], f32, tag="res")
                    nc.vector.tensor_scalar(res, pp, bp_sb[:, ot:ot + 1],
                                            None, add)
                    nc.sync.dma_start(
                        out=out[ot * 128:(ot + 1) * 128, tsl_t], in_=res)
                return f

            pending = []
            for t in range(nt):
                tsl = slice(t * TT, (t + 1) * TT)
                av0 = av_pool.tile([128, TT], f32, tag="av")
                av1 = av_pool.tile([128, TT], f32, tag="av")

                for s in range(ns):
                    st_ps = st_pool.tile([128, 2 * TT], f32, tag="st")
                    ssl = slice(s * 128, (s + 1) * 128)
                    nc.tensor.matmul(st_ps[:, 0:TT], k_sb[0:64, ssl],
                                     q_sb[0:64, tsl], start=True, stop=True)
                    nc.tensor.matmul(st_ps[:, TT:2 * TT], k_sb[64:128, ssl],
                                     q_sb[64:128, tsl], start=True, stop=True)
                    e_sb = e_pool.tile([128, 2 * TT], f32r, tag="e")
                    nc.scalar.activation(e_sb, st_ps, Exp, scale=SCALE)
                    st, sp = (s == 0), (s == ns - 1)
                    nc.tensor.matmul(av0[0:65, :], vt_sb[:, s, 0:65],
                                     e_sb[:, 0:TT], start=st, stop=sp)
                    nc.tensor.matmul(av1[0:65, :], vt_sb[:, s, 65:130],
                                     e_sb[:, TT:2 * TT], start=st, stop=sp)
                    # previous t-tile's projection, spread so single PE ops
                    # with stale deps never block the in-order PE queue
                    if pending and s % 4 == 3:
                        pending.pop(0)()

                # Evacuate the AV psums immediately (unnormalized heads +
                # denominator rows) so their slots free fast -- a slow
                # normalize chain here stalls the in-order PE queue long
                # enough for HAM to re-throttle the clock every t-tile.
                ou = o_pool.tile([128, TT], f32, tag="ou")
                zr0 = z_pool.tile([1, TT], f32r, tag="zr0")
                nc.vector.tensor_copy(ou[0:64, :], av0[0:64, :])
                nc.vector.tensor_copy(zr0, av0[64:65, :])
                zr1 = z_pool.tile([1, TT], f32r, tag="zr1")
                nc.vector.tensor_copy(ou[64:128, :], av1[0:64, :])
                nc.vector.tensor_copy(zr1, av1[64:65, :])

                # ones (x) zrow outer products broadcast each denominator
                # across 64 partitions; psum copied out at once so the shared
                # psum slot frees in ~0.5us, reciprocals run off-path on DVE.
                zbs = z_pool.tile([128, TT], f32, tag="zbs")
                zb0 = pr_pool.tile([64, TT], f32, tag="pp")
                nc.tensor.matmul(zb0, ones_r, zr0, start=True, stop=True)
                nc.vector.tensor_copy(zbs[0:64, :], zb0)
                zb1 = pr_pool.tile([64, TT], f32, tag="pp")
                nc.tensor.matmul(zb1, ones_r, zr1, start=True, stop=True)
                nc.vector.tensor_copy(zbs[64:128, :], zb1)
                rz = z_pool.tile([128, TT], f32, tag="rz")
                nc.vector.reciprocal(rz, zbs)

                o_sb = o_pool.tile([128, TT], f16, tag="o")
                nc.vector.tensor_mul(o_sb[0:64, :], ou[0:64, :], rz[0:64, :])
                nc.vector.tensor_mul(o_sb[64:128, :], ou[64:128, :],
                                     rz[64:128, :])

                # partial output projection, deferred into the next t-tile
                for ot in range(KT):
                    pending.append(mk_proj(o_sb, tsl, ot))
            for f in pending:
                f()

    nc.compile()
    return nc


def _get_nc(l=L):
    if l not in _BUILT:
        _BUILT[l] = _build(l)
    return _BUILT[l]


def _shard_inputs(x, Wq, bq, Wkv, bkv, Wp, bp, l=L):
    x = np.asarray(x, dtype=np.float32)
    Wq = np.asarray(Wq, dtype=np.float32)
    bq = np.asarray(bq, dtype=np.float32)
    Wkv = np.asarray(Wkv, dtype=np.float32)
    bkv = np.asarray(bkv, dtype=np.float32)
    Wp = np.asarray(Wp, dtype=np.float32)
    bp = np.asarray(bp, dtype=np.float32)

    in_maps = []
    for core in range(NCORES):
        b, hp = divmod(core, 4)
        sl = slice(hp * 128, (hp + 1) * 128)
        vsl = slice(C + hp * 128, C + (hp + 1) * 128)
        m = {
            "x": np.ascontiguousarray(x[b].reshape(C, l).astype(np.float16)),
            "wq": np.ascontiguousarray(Wq[sl, :].T.astype(np.float16)),
            "bq": np.ascontiguousarray(bq[sl].reshape(128, 1)),
            "wk": np.ascontiguousarray(Wkv[sl, :].T.astype(np.float16)),
            "bk": np.ascontiguousarray(bkv[sl].reshape(128, 1)),
            "wv": np.ascontiguousarray(Wkv[vsl, :].T.astype(np.float16)),
            "bv": np.ascontiguousarray(bkv[vsl].reshape(128, 1)),
            "wp": np.ascontiguousarray(Wp[:, sl].T.astype(np.float16)),
            "bp": np.ascontiguousarray(
                (bp if hp == 0 else np.zeros_like(bp)).reshape(KT, 128).T),
        }
        in_maps.append(m)
    return in_maps


def _run(in_maps, l=L, trace=False):
    from concourse.bass_utils import run_bass_kernel_spmd
    nc = _get_nc(l)
    return run_bass_kernel_spmd(nc, in_maps, core_ids=list(range(NCORES)),
                                trace=trace)


def kernel(x, Wq, bq, Wkv, bkv, Wp, bp):
    in_maps = _shard_inputs(x, Wq, bq, Wkv, bkv, Wp, bp)
    res = _run(in_maps)
    outs = [res.results[i]["out"] for i in range(NCORES)]
    y = np.stack([outs[0] + outs[1] + outs[2] + outs[3],
                  outs[4] + outs[5] + outs[6] + outs[7]])
    return np.ascontiguousarray(y.reshape(B, C, HH, WW), dtype=np.float32)



# revision 13
# speedup vs baseline: 1.0822x; 1.0822x over previous
"""Trainium2 Bass kernel: MultiHeadAttention over [2, 512, 64, 64] images.

Sharding: 8 cores = (2 batches) x (4 head-pairs). Each core computes 2 of the
8 attention heads for one batch plus a partial output projection over its 128
input channels; the host sums the 4 partial projections per batch and adds the
bias terms that commute with the attention (bv rides through the softmax
weights exactly, bk is a per-query score shift the softmax is invariant to,
bp is a plain output offset).

Per-core schedule (all L=4096 positions), engine-balanced around the Scalar
(ACT) engine which is the hard bottleneck (exp of 33.5M scores at 1
elem/lane/cycle):
  QKV:  Q/K in [c=128, l] f16 (2 heads x 64 dk on partitions), Q pre-scaled
        by softmax_scale/16 (host folds it into Wq), V transposed on the PE
        into VT [s, c] f32r with interleaved ones columns.
  Attn: software-pipelined s-loop per 512-wide t-tile: ST(s+1) is issued to
        the PE queue before AV(s) so the in-order PE never waits on the exp
        of tile s; exp runs split across ACT (cols [0,AW), scale=16) and a
        pair of custom DVE ops (deg-4 Taylor poly p(u) then p^16 by four
        squarings) for cols [AW,1024). The ones rows in VT accumulate the
        softmax denominators in PSUM rows 64/129 for free.
  Tail: per t-tile normalize (approx-reciprocal + ones-outer-product
        broadcast + fused PSUM*PSUM multiply) and the 4 partial-Wp
        projections are deferred into the next t-tile's s-loop, issued
        PE-first at fixed s slots so stale deps never stall the PE queue.
"""

import math
import numpy as np

B, C, HH, WW = 2, 512, 64, 64
L = HH * WW          # 4096
NH, DK = 8, 64
SCALE = 1.0 / math.sqrt(DK)
NCORES = 8

TT = 512             # t-tile width (columns per attention tile)
NT = L // TT         # 8 t-tiles
NS = L // 128        # 32 s-tiles
KT = C // 128        # 4 contraction tiles for projections

AW = 832 # exp columns [0,AW) on ACT, [AW,1024) on DVE custom op

_BUILT = {}
_EXP_OPS = None
DEBUG = False


def _register_exp_ops():
    """Register the two custom DVE ops used for the exp tail share.

    EXP_POLY4_ANT: p = 1 + u*(1 + u*(c2 + u*(c3 + u*c4)))   (8 ALU stages)
    EXP_SQ4_ANT:   out = ((p^2)^2)^2)^2 = p^16               (4 ALU stages)
    With u = score*scale/16 (|u| <~ 0.09 for this problem), p^16 = exp(16u)
    to ~1e-6 relative.
    """
    global _EXP_OPS
    if _EXP_OPS is not None:
        return _EXP_OPS
    import concourse.dve_ops as dvo
    from concourse.dve_spec import Spec, Src0, One, C0, C1, C2, sq, lower
    from concourse.dve_uop import DveOpSpec

    if "EXP_POLY4_ANT" in dvo._SUB_OPCODE_FOR_NAME:
        _EXP_OPS = (dvo.__dict__["EXP_POLY4_ANT"], dvo.__dict__["EXP_SQ4_ANT"])
        return _EXP_OPS

    h = Src0 * C0 + C1
    h = h * Src0 + C2
    h = h * Src0 + One
    p = h * Src0 + One
    spec_poly = Spec(
        body=p,
        reference=lambda in0, in1, s0, s1, imm2: 1.0
        + in0 * (1.0 + in0 * (imm2 + in0 * (s1 + in0 * s0))),
    )
    spec_sq4 = Spec(
        body=sq(sq(sq(sq(Src0)))),
        reference=lambda in0, in1, s0, s1, imm2: (((in0**2) ** 2) ** 2) ** 2,
    )

    ops = []
    for name, spec in (("EXP_POLY4_ANT", spec_poly), ("EXP_SQ4_ANT", spec_sq4)):
        shas = {}
        for ver in ("v3", "v4"):
            uops = lower(spec, ver=ver)
            shas[ver] = DveOpSpec(
                name=name, opcode=0, uops=uops, rd1_en=False
            ).sha(ver)
        op = dvo.DveOp(name, spec, subdim=False, uops_sha=shas)
        dvo.OPS.append(op)
        dvo.CUSTOM_DVE_SPECS[name] = spec
        dvo._SUB_OPCODE_FOR_NAME[name] = dvo._CUSTOM_DVE_ROW_BASE + len(dvo.OPS) - 1
        ops.append(op)
    _EXP_OPS = tuple(ops)
    return _EXP_OPS


def _build(l=L):
    import concourse.bacc as bacc
    import concourse.tile as tile
    import concourse.mybir as mybir
    from concourse.masks import make_identity
    from contextlib import ExitStack

    exp_poly, exp_sq4 = _register_exp_ops()

    nt = l // TT
    ns = l // 128
    f32 = mybir.dt.float32
    f16 = mybir.dt.float16
    f32r = mybir.dt.float32r
    Exp = mybir.ActivationFunctionType.Exp
    add = mybir.AluOpType.add
    mult = mybir.AluOpType.mult

    nc = bacc.Bacc("TRN2", target_bir_lowering=False, debug=False,
                   num_devices=NCORES)

    x = nc.dram_tensor("x", [C, l], f16, kind="ExternalInput").ap()
    wq = nc.dram_tensor("wq", [C, 128], f16, kind="ExternalInput").ap()
    wk = nc.dram_tensor("wk", [C, 128], f16, kind="ExternalInput").ap()
    wv = nc.dram_tensor("wv", [C, 128], f16, kind="ExternalInput").ap()
    bq = nc.dram_tensor("bq", [128, 1], f32, kind="ExternalInput").ap()
    wp = nc.dram_tensor("wp", [128, C], f16, kind="ExternalInput").ap()
    out = nc.dram_tensor("out", [C, l], f32, kind="ExternalOutput").ap()
    dbg = {}
    if DEBUG:
        dbg["z0"] = nc.dram_tensor("dbg_z0", [1, TT], f32,
                                   kind="ExternalOutput").ap()
        dbg["rz0"] = nc.dram_tensor("dbg_rz0", [1, TT], f32,
                                    kind="ExternalOutput").ap()
        dbg["zbs"] = nc.dram_tensor("dbg_zbs", [128, TT], f32,
                                    kind="ExternalOutput").ap()
        dbg["ou"] = nc.dram_tensor("dbg_ou", [128, TT], f32,
                                   kind="ExternalOutput").ap()
        dbg["osb"] = nc.dram_tensor("dbg_osb", [128, TT], f16,
                                    kind="ExternalOutput").ap()
        dbg["e0"] = nc.dram_tensor("dbg_e0", [128, 2 * TT], f32,
                                   kind="ExternalOutput").ap()

    DVW = 1024 - AW  # dve exp column count

    with tile.TileContext(nc) as tc, ExitStack() as ctx:
        persist = ctx.enter_context(tc.tile_pool(name="persist", bufs=1))
        e_pool = ctx.enter_context(tc.tile_pool(name="e", bufs=4))
        p_pool = ctx.enter_context(tc.tile_pool(name="p", bufs=2))
        o_pool = ctx.enter_context(tc.tile_pool(name="o", bufs=2))
        z_pool = ctx.enter_context(tc.tile_pool(name="z", bufs=2))
        res_pool = ctx.enter_context(tc.tile_pool(name="res", bufs=3))

        # ---- weights: fp16 straight from DRAM ----
        w_r = persist.tile([128, 3, KT, 128], f16, tag="wr")
        for i, w in enumerate((wq, wk, wv)):
            for kt in range(KT):
                nc.sync.dma_start(out=w_r[:, i, kt, :],
                                  in_=w[kt * 128:(kt + 1) * 128, :])
        wp_r = persist.tile([128, C], f16, tag="wpr")
        for kt in range(KT):
            nc.sync.dma_start(out=wp_r[:, kt * 128:(kt + 1) * 128],
                              in_=wp[:, kt * 128:(kt + 1) * 128])

        bq_sb = persist.tile([128, 1], f32, tag="bq")
        nc.sync.dma_start(out=bq_sb, in_=bq)

        ident = persist.tile([128, 128], f16, tag="ident")
        make_identity(nc, ident)

        ones_r = persist.tile([1, 64], f32, tag="onesr")
        nc.vector.memset(ones_r, 1.0)

        # ---- persistent activations ----
        q_sb = persist.tile([128, l], f16, tag="q")
        k_sb = persist.tile([128, l], f16, tag="k")
        vt_sb = persist.tile([128, ns, 130], f32r, tag="vt")
        # interleaved ones columns -> softmax denominator rows in AV psum
        ones_col = persist.tile([128, ns, 1], f32, tag="onescol")
        nc.vector.memset(ones_col, 1.0)
        nc.vector.tensor_copy(vt_sb[:, :, 64:65], ones_col)
        nc.vector.tensor_copy(vt_sb[:, :, 129:130], ones_col)

        # ================= QKV projections + V transpose =================
        with ExitStack() as qctx:
            x_pool = qctx.enter_context(tc.tile_pool(name="xs", bufs=3))
            qkv_ps = qctx.enter_context(
                tc.tile_pool(name="qkvps", bufs=2, space="PSUM"))
            v_pool = qctx.enter_context(tc.tile_pool(name="vsb", bufs=2))

            for n in range(nt):
                nsl = slice(n * TT, (n + 1) * TT)
                x_r = x_pool.tile([128, KT, TT], f16, tag="x")
                for kt in range(KT):
                    half = TT // 2
                    for hh in range(2):
                        csl = slice(hh * half, (hh + 1) * half)
                        nc.sync.dma_start(
                            out=x_r[:, kt, csl],
                            in_=x[kt * 128:(kt + 1) * 128,
                                  n * TT + hh * half:n * TT + (hh + 1) * half])

                q_ps = qkv_ps.tile([128, TT], f32, tag="qps")
                k_ps = qkv_ps.tile([128, TT], f32, tag="kps")
                v_ps = qkv_ps.tile([128, TT], f32, tag="vps")
                for wi, ps in enumerate((q_ps, k_ps, v_ps)):
                    for kt in range(KT):
                        nc.tensor.matmul(ps, w_r[:, wi, kt, :], x_r[:, kt, :],
                                         start=(kt == 0), stop=(kt == KT - 1))

                nc.vector.tensor_scalar(q_sb[:, nsl], q_ps,
                                        bq_sb, None, add)
                nc.scalar.copy(k_sb[:, nsl], k_ps)
                v_sb = v_pool.tile([128, TT], f16, tag="v")
                nc.vector.tensor_copy(v_sb, v_ps)

                # transpose V tile: 4 PE transposes -> [s, c] in psum
                tp = qkv_ps.tile([128, TT], f16, tag="tp")
                for j in range(4):
                    nc.tensor.transpose(tp[:, j * 128:(j + 1) * 128],
                                        v_sb[:, j * 128:(j + 1) * 128], ident)
                tp_v = tp.rearrange("p (j c) -> p j c", j=4)
                ssl = slice(4 * n, 4 * n + 4)
                nc.vector.tensor_copy(vt_sb[:, ssl, 0:64], tp_v[:, :, 0:64])
                nc.vector.tensor_copy(vt_sb[:, ssl, 65:129], tp_v[:, :, 64:128])

        # ========================= attention =========================
        with ExitStack() as actx:
            st_pool = actx.enter_context(
                tc.tile_pool(name="stps", bufs=2, space="PSUM"))
            av_pool = actx.enter_context(
                tc.tile_pool(name="avps", bufs=3, space="PSUM"))
            pr_pool = actx.enter_context(
                tc.tile_pool(name="prps", bufs=1, space="PSUM"))

            # Deferred tail work from the previous t-tile. The reciprocal +
            # denominator-broadcast chain is issued right at the t-boundary
            # (rcp first in the DVE queue, zbmm after two ST prefetches in
            # the PE queue); normalize muls and the partial projections are
            # spread over the next t-tile's s-loop at fixed slots, PE work
            # issued first within an iteration so stale deps never block
            # younger PE instructions in the in-order queue.
            def make_tail(av0, av1, tsl, dbg_t=False):
                rz0 = z_pool.tile([1, TT], f32, tag="rz0")
                rz1 = z_pool.tile([1, TT], f32, tag="rz1")
                zb = pr_pool.tile([128, TT], f32, tag="pp")
                zbs = z_pool.tile([128, TT], f32, tag="zbs")
                ou = z_pool.tile([128, TT], f32, tag="ou")
                o_sb = o_pool.tile([128, TT], f16, tag="o")

                zr0 = z_pool.tile([1, TT], f32, tag="zr0")
                zr1 = z_pool.tile([1, TT], f32, tag="zr1")

                def rcp():
                    # custom-DVE ops drop the partition offset of their in0,
                    # so stage the z rows to partition-0 tiles first
                    nc.vector.tensor_copy(zr0, av0[64:65, :])
                    nc.vector.tensor_copy(zr1, av1[64:65, :])
                    nc.vector.reciprocal_approx_fast(out=rz0, in_=zr0)
                    nc.vector.reciprocal_approx_fast(out=rz1, in_=zr1)
                    if dbg_t:
                        nc.sync.dma_start(out=dbg["z0"], in_=zr0)
                        nc.sync.dma_start(out=dbg["rz0"], in_=rz0)

                def zbmm():
                    nc.tensor.matmul(zb[0:64, :], ones_r,
                                     rz0, start=True, stop=True)
                    nc.tensor.matmul(zb[64:128, :], ones_r,
                                     rz1, start=True, stop=True)

                def ev0():
                    nc.vector.tensor_copy(ou[0:64, :], av0[0:64, :])

                def ev1():
                    nc.vector.tensor_copy(ou[64:128, :], av1[0:64, :])

                def zbcp():
                    nc.vector.tensor_copy(zbs, zb)

                def mul():
                    nc.vector.tensor_mul(o_sb, ou, zbs)
                    if dbg_t:
                        nc.sync.dma_start(out=dbg["zbs"], in_=zbs)
                        nc.sync.dma_start(out=dbg["ou"], in_=ou)
                        nc.sync.dma_start(out=dbg["osb"], in_=o_sb)

                work = [(0, False, ev0), (1, False, ev1), (2, False, zbcp),
                        (3, False, mul)]
                for i, ot in enumerate(range(KT)):
                    pp = pr_pool.tile([128, TT], f32, tag="pp")

                    def mmfn(pp=pp, ot=ot):
                        nc.tensor.matmul(pp, wp_r[:, ot * 128:(ot + 1) * 128],
                                         o_sb, start=True, stop=True)

                    def cpfn(pp=pp, ot=ot):
                        res = res_pool.tile([128, TT], f32, tag="res")
                        nc.vector.tensor_copy(res, pp)
                        nc.sync.dma_start(
                            out=out[ot * 128:(ot + 1) * 128, tsl], in_=res)

                    work.append((8 + 4 * i, True, mmfn))
                    work.append((10 + 4 * i, False, cpfn))
                return rcp, zbmm, work

            tail_rcp, tail_zbmm, tail = None, None, []
            for t in range(nt):
                tsl = slice(t * TT, (t + 1) * TT)
                av0 = av_pool.tile([128, TT], f32, tag="av")
                av1 = av_pool.tile([128, TT], f32, tag="av")

                def issue_st(s, tsl=tsl):
                    ssl = slice(s * 128, (s + 1) * 128)
                    st_ps = st_pool.tile([128, 2 * TT], f32, tag="st")
                    nc.tensor.matmul(st_ps[:, 0:TT], k_sb[0:64, ssl],
                                     q_sb[0:64, tsl], start=True, stop=True)
                    nc.tensor.matmul(st_ps[:, TT:2 * TT], k_sb[64:128, ssl],
                                     q_sb[64:128, tsl], start=True, stop=True)
                    return st_ps

                # t-boundary: two ST prefetches, then the old tile's
                # normalize chain heads both queues
                st_tiles = {0: issue_st(0), 1: issue_st(1)}
                if tail_rcp is not None:
                    tail_rcp()
                    tail_zbmm()

                for s in range(ns):
                    for slot, is_pe, fn in tail:
                        if slot == s and is_pe:
                            fn()
                    if s + 2 < ns:
                        st_tiles[s + 2] = issue_st(s + 2)
                    st_ps = st_tiles.pop(s)

                    e_sb = e_pool.tile([128, 2 * TT], f32r, tag="e")
                    nc.scalar.activation(e_sb[:, 0:AW], st_ps[:, 0:AW], Exp,
                                         scale=16.0)
                    if DVW:
                        p_sb = p_pool.tile([128, DVW], f32, tag="p")
                        nc.vector._custom_dve(exp_poly, out=p_sb,
                                              in0=st_ps[:, AW:2 * TT],
                                              s0=1.0 / 24.0, s1=1.0 / 6.0,
                                              imm2=0.5)
                        nc.vector._custom_dve(exp_sq4,
                                              out=e_sb[:, AW:2 * TT], in0=p_sb)

                    if DEBUG and t == 0 and s == 0:
                        nc.sync.dma_start(out=dbg["e0"],
                                          in_=e_sb.bitcast(f32))
                    stt, spp = (s == 0), (s == ns - 1)
                    nc.tensor.matmul(av0[0:65, :], vt_sb[:, s, 0:65],
                                     e_sb[:, 0:TT], start=stt, stop=spp)
                    nc.tensor.matmul(av1[0:65, :], vt_sb[:, s, 65:130],
                                     e_sb[:, TT:2 * TT], start=stt, stop=spp)

                    for slot, is_pe, fn in tail:
                        if slot == s and not is_pe:
                            fn()
                tail_rcp, tail_zbmm, tail = make_tail(av0, av1, tsl,
                                                      dbg_t=(DEBUG and t == 0))

            # epilogue: flush the final t-tile's tail work
            tail_rcp()
            tail_zbmm()
            for slot, is_pe, fn in sorted(tail, key=lambda w: w[0]):
                fn()

    nc.compile()
    return nc


def _get_nc(l=L):
    if l not in _BUILT:
        _BUILT[l] = _build(l)
    return _BUILT[l]


def _shard_inputs(x, Wq, bq, Wkv, bkv, Wp, bp, l=L):
    x = np.asarray(x, dtype=np.float32)
    Wq = np.asarray(Wq, dtype=np.float32)
    bq = np.asarray(bq, dtype=np.float32)
    Wkv = np.asarray(Wkv, dtype=np.float32)
    bkv = np.asarray(bkv, dtype=np.float32)
    Wp = np.asarray(Wp, dtype=np.float32)
    bp = np.asarray(bp, dtype=np.float32)

    s16 = SCALE / 16.0
    in_maps = []
    for core in range(NCORES):
        b, hp = divmod(core, 4)
        sl = slice(hp * 128, (hp + 1) * 128)
        vsl = slice(C + hp * 128, C + (hp + 1) * 128)
        m = {
            "x": np.ascontiguousarray(x[b].reshape(C, l).astype(np.float16)),
            "wq": np.ascontiguousarray(
                (Wq[sl, :] * s16).T.astype(np.float16)),
            "bq": np.ascontiguousarray(
                (bq[sl] * s16).reshape(128, 1).astype(np.float32)),
            "wk": np.ascontiguousarray(Wkv[sl, :].T.astype(np.float16)),
            "wv": np.ascontiguousarray(Wkv[vsl, :].T.astype(np.float16)),
            "wp": np.ascontiguousarray(Wp[:, sl].T.astype(np.float16)),
        }
        in_maps.append(m)
    return in_maps


def _host_const(Wkv, bkv, Wp, bp):
    """Bias terms folded out of the kernel: out += Wp @ bv + bp (exact)."""
    Wkv = np.asarray(Wkv, dtype=np.float64)
    bkv = np.asarray(bkv, dtype=np.float64)
    Wp = np.asarray(Wp, dtype=np.float64)
    bp = np.asarray(bp, dtype=np.float64)
    bv = bkv[C:]
    return (Wp @ bv + bp).astype(np.float32)


def _run(in_maps, l=L, trace=False):
    from concourse.bass_utils import run_bass_kernel_spmd
    nc = _get_nc(l)
    return run_bass_kernel_spmd(nc, in_maps, core_ids=list(range(NCORES)),
                                trace=trace)


def kernel(x, Wq, bq, Wkv, bkv, Wp, bp):
    in_maps = _shard_inputs(x, Wq, bq, Wkv, bkv, Wp, bp)
    res = _run(in_maps)
    outs = [res.results[i]["out"] for i in range(NCORES)]
    cv = _host_const(Wkv, bkv, Wp, bp)[None, :, None]
    y = np.stack([outs[0] + outs[1] + outs[2] + outs[3],
                  outs[4] + outs[5] + outs[6] + outs[7]]) + cv
    return np.ascontiguousarray(y.reshape(B, C, HH, WW), dtype=np.float32)


# revision 14
# speedup vs baseline: 1.1028x; 1.0190x over previous
"""Trainium2 Bass kernel: MultiHeadAttention over [2, 512, 64, 64] images.

Sharding: 8 cores = (2 batches) x (4 head-pairs). Each core computes 2 of the
8 attention heads for one batch plus a partial output projection over its 128
input channels; the host sums the 4 partial projections per batch and adds the
bias terms that commute with the attention (bv rides through the softmax
weights exactly, bk is a per-query score shift the softmax is invariant to,
bp is a plain output offset).

Per-core schedule (all L=4096 positions), engine-balanced around the Scalar
(ACT) engine which is the hard bottleneck (exp of 33.5M scores at 1
elem/lane/cycle):
  QKV:  Q/K in [c=128, l] f16 (2 heads x 64 dk on partitions), Q pre-scaled
        by softmax_scale/16 (host folds it into Wq), V transposed on the PE
        into VT [s, c] f32r with interleaved ones columns.
  Attn: software-pipelined s-loop per 512-wide t-tile: ST(s+1) is issued to
        the PE queue before AV(s) so the in-order PE never waits on the exp
        of tile s; exp runs split across ACT (cols [0,AW), scale=16) and a
        pair of custom DVE ops (deg-4 Taylor poly p(u) then p^16 by four
        squarings) for cols [AW,1024). The ones rows in VT accumulate the
        softmax denominators in PSUM rows 64/129 for free.
  Tail: per t-tile normalize (approx-reciprocal + ones-outer-product
        broadcast + fused PSUM*PSUM multiply) and the 4 partial-Wp
        projections are deferred into the next t-tile's s-loop, issued
        PE-first at fixed s slots so stale deps never stall the PE queue.
"""

import math
import numpy as np

B, C, HH, WW = 2, 512, 64, 64
L = HH * WW          # 4096
NH, DK = 8, 64
SCALE = 1.0 / math.sqrt(DK)
NCORES = 8

TT = 512             # t-tile width (columns per attention tile)
NT = L // TT         # 8 t-tiles
NS = L // 128        # 32 s-tiles
KT = C // 128        # 4 contraction tiles for projections

AW = 832 # exp columns [0,AW) on ACT, [AW,1024) on DVE custom op

_BUILT = {}
_EXP_OPS = None
DEBUG = False


def _register_exp_ops():
    """Register the two custom DVE ops used for the exp tail share.

    EXP_POLY4_ANT: p = 1 + u*(1 + u*(c2 + u*(c3 + u*c4)))   (8 ALU stages)
    EXP_SQ4_ANT:   out = ((p^2)^2)^2)^2 = p^16               (4 ALU stages)
    With u = score*scale/16 (|u| <~ 0.09 for this problem), p^16 = exp(16u)
    to ~1e-6 relative.
    """
    global _EXP_OPS
    if _EXP_OPS is not None:
        return _EXP_OPS
    import concourse.dve_ops as dvo
    from concourse.dve_spec import Spec, Src0, One, C0, C1, C2, sq, lower
    from concourse.dve_uop import DveOpSpec

    if "EXP_POLY4_ANT" in dvo._SUB_OPCODE_FOR_NAME:
        _EXP_OPS = (dvo.__dict__["EXP_POLY4_ANT"], dvo.__dict__["EXP_SQ4_ANT"])
        return _EXP_OPS

    h = Src0 * C0 + C1
    h = h * Src0 + C2
    h = h * Src0 + One
    p = h * Src0 + One
    spec_poly = Spec(
        body=p,
        reference=lambda in0, in1, s0, s1, imm2: 1.0
        + in0 * (1.0 + in0 * (imm2 + in0 * (s1 + in0 * s0))),
    )
    spec_sq4 = Spec(
        body=sq(sq(sq(sq(Src0)))),
        reference=lambda in0, in1, s0, s1, imm2: (((in0**2) ** 2) ** 2) ** 2,
    )

    ops = []
    for name, spec in (("EXP_POLY4_ANT", spec_poly), ("EXP_SQ4_ANT", spec_sq4)):
        shas = {}
        for ver in ("v3", "v4"):
            uops = lower(spec, ver=ver)
            shas[ver] = DveOpSpec(
                name=name, opcode=0, uops=uops, rd1_en=False
            ).sha(ver)
        op = dvo.DveOp(name, spec, subdim=False, uops_sha=shas)
        dvo.OPS.append(op)
        dvo.CUSTOM_DVE_SPECS[name] = spec
        dvo._SUB_OPCODE_FOR_NAME[name] = dvo._CUSTOM_DVE_ROW_BASE + len(dvo.OPS) - 1
        ops.append(op)
    _EXP_OPS = tuple(ops)
    return _EXP_OPS


def _build(l=L):
    import concourse.bacc as bacc
    import concourse.tile as tile
    import concourse.mybir as mybir
    from concourse.masks import make_identity
    from contextlib import ExitStack

    exp_poly, exp_sq4 = _register_exp_ops()

    nt = l // TT
    ns = l // 128
    f32 = mybir.dt.float32
    f16 = mybir.dt.float16
    f32r = mybir.dt.float32r
    Exp = mybir.ActivationFunctionType.Exp
    add = mybir.AluOpType.add
    mult = mybir.AluOpType.mult

    nc = bacc.Bacc("TRN2", target_bir_lowering=False, debug=False,
                   num_devices=NCORES)

    x = nc.dram_tensor("x", [C, l], f16, kind="ExternalInput").ap()
    # wpack: [128, 12*128 (wq|wk|wv kt-tiles) + 512 (wp)] fp16
    wpack = nc.dram_tensor("wpack", [128, 2048], f16, kind="ExternalInput").ap()
    bq = nc.dram_tensor("bq", [128, 1], f32, kind="ExternalInput").ap()
    out = nc.dram_tensor("out", [C, l], f32, kind="ExternalOutput").ap()
    dbg = {}
    if DEBUG:
        dbg["z0"] = nc.dram_tensor("dbg_z0", [1, TT], f32,
                                   kind="ExternalOutput").ap()
        dbg["rz0"] = nc.dram_tensor("dbg_rz0", [1, TT], f32,
                                    kind="ExternalOutput").ap()
        dbg["zbs"] = nc.dram_tensor("dbg_zbs", [128, TT], f32,
                                    kind="ExternalOutput").ap()
        dbg["ou"] = nc.dram_tensor("dbg_ou", [128, TT], f32,
                                   kind="ExternalOutput").ap()
        dbg["osb"] = nc.dram_tensor("dbg_osb", [128, TT], f16,
                                    kind="ExternalOutput").ap()
        dbg["e0"] = nc.dram_tensor("dbg_e0", [128, 2 * TT], f32,
                                   kind="ExternalOutput").ap()

    DVW = 1024 - AW  # dve exp column count

    with tile.TileContext(nc) as tc, ExitStack() as ctx:
        persist = ctx.enter_context(tc.tile_pool(name="persist", bufs=1))
        e_pool = ctx.enter_context(tc.tile_pool(name="e", bufs=6))
        p_pool = ctx.enter_context(tc.tile_pool(name="p", bufs=2))
        o_pool = ctx.enter_context(tc.tile_pool(name="o", bufs=2))
        z_pool = ctx.enter_context(tc.tile_pool(name="z", bufs=2))
        res_pool = ctx.enter_context(tc.tile_pool(name="res", bufs=3))

        # ---- weights: one packed fp16 block, 4 chunked DMAs ----
        wall = persist.tile([128, 2048], f16, tag="wall")
        for c4 in range(4):
            nc.sync.dma_start(out=wall[:, c4 * 512:(c4 + 1) * 512],
                              in_=wpack[:, c4 * 512:(c4 + 1) * 512])
        w_r = wall[:, 0:1536].rearrange("p (i kt c) -> p i kt c", i=3, kt=KT)
        wp_r = wall[:, 1536:2048]

        bq_sb = persist.tile([128, 1], f32, tag="bq")
        nc.sync.dma_start(out=bq_sb, in_=bq)

        ident = persist.tile([128, 128], f16, tag="ident")
        make_identity(nc, ident)

        ones_r = persist.tile([1, 64], f32, tag="onesr")
        nc.vector.memset(ones_r, 1.0)

        # ---- persistent activations ----
        q_sb = persist.tile([128, l], f16, tag="q")
        k_sb = persist.tile([128, l], f16, tag="k")
        vt_sb = persist.tile([128, ns, 130], f32r, tag="vt")
        # interleaved ones columns -> softmax denominator rows in AV psum
        ones_col = persist.tile([128, ns, 1], f32, tag="onescol")
        nc.vector.memset(ones_col, 1.0)
        nc.vector.tensor_copy(vt_sb[:, :, 64:65], ones_col)
        nc.vector.tensor_copy(vt_sb[:, :, 129:130], ones_col)

        # ================= QKV projections + V transpose =================
        with ExitStack() as qctx:
            x_pool = qctx.enter_context(tc.tile_pool(name="xs", bufs=3))
            qkv_ps = qctx.enter_context(
                tc.tile_pool(name="qkvps", bufs=2, space="PSUM"))
            v_pool = qctx.enter_context(tc.tile_pool(name="vsb", bufs=2))

            for n in range(nt):
                nsl = slice(n * TT, (n + 1) * TT)
                x_r = x_pool.tile([128, KT, TT], f16, tag="x")
                for kt in range(KT):
                    half = TT // 2
                    for hh in range(2):
                        csl = slice(hh * half, (hh + 1) * half)
                        nc.gpsimd.dma_start(
                            out=x_r[:, kt, csl],
                            in_=x[kt * 128:(kt + 1) * 128,
                                  n * TT + hh * half:n * TT + (hh + 1) * half])

                q_ps = qkv_ps.tile([128, TT], f32, tag="qps")
                k_ps = qkv_ps.tile([128, TT], f32, tag="kps")
                v_ps = qkv_ps.tile([128, TT], f32, tag="vps")
                for wi, ps in enumerate((q_ps, k_ps, v_ps)):
                    for kt in range(KT):
                        nc.tensor.matmul(ps, w_r[:, wi, kt, :], x_r[:, kt, :],
                                         start=(kt == 0), stop=(kt == KT - 1))

                nc.vector.tensor_scalar(q_sb[:, nsl], q_ps,
                                        bq_sb, None, add)
                nc.scalar.copy(k_sb[:, nsl], k_ps)
                v_sb = v_pool.tile([128, TT], f16, tag="v")
                nc.vector.tensor_copy(v_sb, v_ps)

                # transpose V tile: 4 PE transposes -> [s, c] in psum
                tp = qkv_ps.tile([128, TT], f16, tag="tp")
                for j in range(4):
                    nc.tensor.transpose(tp[:, j * 128:(j + 1) * 128],
                                        v_sb[:, j * 128:(j + 1) * 128], ident)
                tp_v = tp.rearrange("p (j c) -> p j c", j=4)
                ssl = slice(4 * n, 4 * n + 4)
                nc.vector.tensor_copy(vt_sb[:, ssl, 0:64], tp_v[:, :, 0:64])
                nc.vector.tensor_copy(vt_sb[:, ssl, 65:129], tp_v[:, :, 64:128])

        # ========================= attention =========================
        with ExitStack() as actx:
            st_pool = actx.enter_context(
                tc.tile_pool(name="stps", bufs=2, space="PSUM"))
            av_pool = actx.enter_context(
                tc.tile_pool(name="avps", bufs=3, space="PSUM"))
            pr_pool = actx.enter_context(
                tc.tile_pool(name="prps", bufs=1, space="PSUM"))

            # Deferred tail work from the previous t-tile. The reciprocal +
            # denominator-broadcast chain is issued right at the t-boundary
            # (rcp first in the DVE queue, zbmm after two ST prefetches in
            # the PE queue); normalize muls and the partial projections are
            # spread over the next t-tile's s-loop at fixed slots, PE work
            # issued first within an iteration so stale deps never block
            # younger PE instructions in the in-order queue.
            def make_tail(av0, av1, tsl, dbg_t=False):
                rz0 = z_pool.tile([1, TT], f32, tag="rz0")
                rz1 = z_pool.tile([1, TT], f32, tag="rz1")
                zb = pr_pool.tile([128, TT], f32, tag="pp")
                zbs = z_pool.tile([128, TT], f32, tag="zbs")
                ou = z_pool.tile([128, TT], f32, tag="ou")
                o_sb = o_pool.tile([128, TT], f16, tag="o")

                zr0 = z_pool.tile([1, TT], f32, tag="zr0")
                zr1 = z_pool.tile([1, TT], f32, tag="zr1")

                def rcp():
                    # custom-DVE ops drop the partition offset of their in0,
                    # so stage the z rows to partition-0 tiles first
                    nc.vector.tensor_copy(zr0, av0[64:65, :])
                    nc.vector.reciprocal_approx_fast(out=rz0, in_=zr0)
                    nc.vector.tensor_copy(zr1, av1[64:65, :])
                    nc.vector.reciprocal_approx_fast(out=rz1, in_=zr1)
                    if dbg_t:
                        nc.sync.dma_start(out=dbg["z0"], in_=zr0)
                        nc.sync.dma_start(out=dbg["rz0"], in_=rz0)

                def zbmm():
                    nc.tensor.matmul(zb[0:64, :], ones_r,
                                     rz0, start=True, stop=True)
                    nc.tensor.matmul(zb[64:128, :], ones_r,
                                     rz1, start=True, stop=True)

                def ev0():
                    nc.vector.tensor_copy(ou[0:64, :], av0[0:64, :])

                def ev1():
                    nc.vector.tensor_copy(ou[64:128, :], av1[0:64, :])

                def zbcp():
                    nc.vector.tensor_copy(zbs, zb)

                def mul():
                    nc.vector.tensor_mul(o_sb, ou, zbs)
                    if dbg_t:
                        nc.sync.dma_start(out=dbg["zbs"], in_=zbs)
                        nc.sync.dma_start(out=dbg["ou"], in_=ou)
                        nc.sync.dma_start(out=dbg["osb"], in_=o_sb)

                work = [(0, False, ev0), (1, False, ev1), (2, True, zbmm),
                        (3, False, zbcp), (4, False, mul)]
                for i, ot in enumerate(range(KT)):
                    pp = pr_pool.tile([128, TT], f32, tag="pp")

                    def mmfn(pp=pp, ot=ot):
                        nc.tensor.matmul(pp, wp_r[:, ot * 128:(ot + 1) * 128],
                                         o_sb, start=True, stop=True)

                    def cpfn(pp=pp, ot=ot):
                        res = res_pool.tile([128, TT], f32, tag="res")
                        nc.vector.tensor_copy(res, pp)
                        nc.gpsimd.dma_start(
                            out=out[ot * 128:(ot + 1) * 128, tsl], in_=res)

                    work.append((8 + 4 * i, True, mmfn))
                    work.append((10 + 4 * i, False, cpfn))
                return rcp, work

            tail_rcp, tail = None, []
            for t in range(nt):
                tsl = slice(t * TT, (t + 1) * TT)
                av0 = av_pool.tile([128, TT], f32, tag="av")
                av1 = av_pool.tile([128, TT], f32, tag="av")

                def issue_st(s, tsl=tsl):
                    ssl = slice(s * 128, (s + 1) * 128)
                    st_ps = st_pool.tile([128, 2 * TT], f32, tag="st")
                    nc.tensor.matmul(st_ps[:, 0:TT], k_sb[0:64, ssl],
                                     q_sb[0:64, tsl], start=True, stop=True)
                    nc.tensor.matmul(st_ps[:, TT:2 * TT], k_sb[64:128, ssl],
                                     q_sb[64:128, tsl], start=True, stop=True)
                    return st_ps

                # t-boundary: two ST prefetches, then the old tile's
                # normalize chain heads both queues
                st_tiles = {0: issue_st(0), 1: issue_st(1)}
                h1q, e_tiles = [], {}
                if tail_rcp is not None:
                    tail_rcp()

                for s in range(ns):
                    for slot, is_pe, fn in tail:
                        if slot == s and is_pe:
                            fn()
                    if s + 2 < ns:
                        st_tiles[s + 2] = issue_st(s + 2)
                    st_ps = st_tiles.pop(s)

                    e_sb = e_pool.tile([128, 2 * TT], f32r, tag="e")
                    nc.scalar.activation(e_sb[:, 0:AW], st_ps[:, 0:AW], Exp,
                                         scale=16.0)
                    if DVW:
                        p_sb = p_pool.tile([128, DVW], f32, tag="p")
                        nc.vector._custom_dve(exp_poly, out=p_sb,
                                              in0=st_ps[:, AW:2 * TT],
                                              s0=1.0 / 24.0, s1=1.0 / 6.0,
                                              imm2=0.5)
                        nc.vector._custom_dve(exp_sq4,
                                              out=e_sb[:, AW:2 * TT], in0=p_sb)

                    if DEBUG and t == 0 and s == 0:
                        nc.sync.dma_start(out=dbg["e0"],
                                          in_=e_sb.bitcast(f32))
                    nc.tensor.matmul(av0[0:65, :], vt_sb[:, s, 0:65],
                                     e_sb[:, 0:TT], start=(s == 0),
                                     stop=(s == ns - 1))
                    # h1 AV stream runs ~4 s-tiles behind h0 (PSUM accumulate
                    # order is free) so the first h1 write of a t-tile lands
                    # after the previous tile's av slot is drained
                    e_tiles[s] = e_sb
                    h1q.append(s)
                    npop = 2 if 4 <= s < 8 else (1 if s >= 8 else 0)
                    for _ in range(npop):
                        j = h1q.pop(0)
                        ej = e_tiles.pop(j)
                        nc.tensor.matmul(av1[0:65, :], vt_sb[:, j, 65:130],
                                         ej[:, TT:2 * TT], start=(j == 0),
                                         stop=(j == ns - 1))

                    for slot, is_pe, fn in tail:
                        if slot == s and not is_pe:
                            fn()
                tail_rcp, tail = make_tail(av0, av1, tsl,
                                           dbg_t=(DEBUG and t == 0))

            # epilogue: flush the final t-tile's tail work
            tail_rcp()
            for slot, is_pe, fn in sorted(tail, key=lambda w: w[0]):
                fn()

    nc.compile()
    return nc


def _get_nc(l=L):
    if l not in _BUILT:
        _BUILT[l] = _build(l)
    return _BUILT[l]


def _shard_inputs(x, Wq, bq, Wkv, bkv, Wp, bp, l=L):
    x = np.asarray(x, dtype=np.float32)
    Wq = np.asarray(Wq, dtype=np.float32)
    bq = np.asarray(bq, dtype=np.float32)
    Wkv = np.asarray(Wkv, dtype=np.float32)
    bkv = np.asarray(bkv, dtype=np.float32)
    Wp = np.asarray(Wp, dtype=np.float32)
    bp = np.asarray(bp, dtype=np.float32)

    s16 = SCALE / 16.0
    in_maps = []
    for core in range(NCORES):
        b, hp = divmod(core, 4)
        sl = slice(hp * 128, (hp + 1) * 128)
        vsl = slice(C + hp * 128, C + (hp + 1) * 128)
        wq_t = (Wq[sl, :] * s16).T.astype(np.float16)      # [C, 128]
        wk_t = Wkv[sl, :].T.astype(np.float16)
        wv_t = Wkv[vsl, :].T.astype(np.float16)
        wp_t = Wp[:, sl].T.astype(np.float16)               # [128, C]
        wpack = np.empty((128, 2048), dtype=np.float16)
        for i, w in enumerate((wq_t, wk_t, wv_t)):
            for kt in range(KT):
                wpack[:, (i * KT + kt) * 128:(i * KT + kt + 1) * 128] = \
                    w[kt * 128:(kt + 1) * 128, :]
        wpack[:, 1536:2048] = wp_t
        m = {
            "x": np.ascontiguousarray(x[b].reshape(C, l).astype(np.float16)),
            "wpack": np.ascontiguousarray(wpack),
            "bq": np.ascontiguousarray(
                (bq[sl] * s16).reshape(128, 1).astype(np.float32)),
        }
        in_maps.append(m)
    return in_maps


def _host_const(Wkv, bkv, Wp, bp):
    """Bias terms folded out of the kernel: out += Wp @ bv + bp (exact)."""
    Wkv = np.asarray(Wkv, dtype=np.float64)
    bkv = np.asarray(bkv, dtype=np.float64)
    Wp = np.asarray(Wp, dtype=np.float64)
    bp = np.asarray(bp, dtype=np.float64)
    bv = bkv[C:]
    return (Wp @ bv + bp).astype(np.float32)


def _run(in_maps, l=L, trace=False):
    from concourse.bass_utils import run_bass_kernel_spmd
    nc = _get_nc(l)
    return run_bass_kernel_spmd(nc, in_maps, core_ids=list(range(NCORES)),
                                trace=trace)


def kernel(x, Wq, bq, Wkv, bkv, Wp, bp):
    in_maps = _shard_inputs(x, Wq, bq, Wkv, bkv, Wp, bp)
    res = _run(in_maps)
    outs = [res.results[i]["out"] for i in range(NCORES)]
    cv = _host_const(Wkv, bkv, Wp, bp)[None, :, None]
    y = np.stack([outs[0] + outs[1] + outs[2] + outs[3],
                  outs[4] + outs[5] + outs[6] + outs[7]]) + cv
    return np.ascontiguousarray(y.reshape(B, C, HH, WW), dtype=np.float32)


# revision 16
# speedup vs baseline: 1.1457x; 1.0389x over previous
"""Trainium2 Bass kernel: MultiHeadAttention over [2, 512, 64, 64] images.

Sharding: 8 cores = (2 batches) x (4 head-pairs). Each core computes 2 of the
8 attention heads for one batch plus a partial output projection over its 128
input channels; the host sums the 4 partial projections per batch and adds the
bias terms that commute with the attention (bv rides through the softmax
weights exactly, bk is a per-query score shift the softmax is invariant to,
bp is a plain output offset).

Per-core schedule (all L=4096 positions), engine-balanced around the Scalar
(ACT) engine which is the hard bottleneck (exp of 33.5M scores at 1
elem/lane/cycle):
  QKV:  Q/K in [c=128, l] f16 (2 heads x 64 dk on partitions), Q pre-scaled
        by softmax_scale/16 (host folds it into Wq), V transposed on the PE
        into VT [s, c] f32r with interleaved ones columns.
  Attn: software-pipelined s-loop per 512-wide t-tile: ST(s+1) is issued to
        the PE queue before AV(s) so the in-order PE never waits on the exp
        of tile s; exp runs split across ACT (cols [0,AW), scale=16) and a
        pair of custom DVE ops (deg-4 Taylor poly p(u) then p^16 by four
        squarings) for cols [AW,1024). The ones rows in VT accumulate the
        softmax denominators in PSUM rows 64/129 for free.
  Tail: per t-tile normalize (approx-reciprocal + ones-outer-product
        broadcast + fused PSUM*PSUM multiply) and the 4 partial-Wp
        projections are deferred into the next t-tile's s-loop, issued
        PE-first at fixed s slots so stale deps never stall the PE queue.
"""

import math
import numpy as np

B, C, HH, WW = 2, 512, 64, 64
L = HH * WW          # 4096
NH, DK = 8, 64
SCALE = 1.0 / math.sqrt(DK)
NCORES = 8

TT = 512             # t-tile width (columns per attention tile)
NT = L // TT         # 8 t-tiles
NS = L // 128        # 32 s-tiles
KT = C // 128        # 4 contraction tiles for projections

AW = 832 # exp columns [0,AW) on ACT, [AW,1024) on DVE custom op

_BUILT = {}
_EXP_OPS = None
DEBUG = False


def _register_exp_ops():
    """Register the two custom DVE ops used for the exp tail share.

    EXP_POLY4_ANT: p = 1 + u*(1 + u*(c2 + u*(c3 + u*c4)))   (8 ALU stages)
    EXP_SQ4_ANT:   out = ((p^2)^2)^2)^2 = p^16               (4 ALU stages)
    With u = score*scale/16 (|u| <~ 0.09 for this problem), p^16 = exp(16u)
    to ~1e-6 relative.
    """
    global _EXP_OPS
    if _EXP_OPS is not None:
        return _EXP_OPS
    import concourse.dve_ops as dvo
    from concourse.dve_spec import Spec, Src0, One, C0, C1, C2, sq, lower
    from concourse.dve_uop import DveOpSpec

    if "EXP_POLY4_ANT" in dvo._SUB_OPCODE_FOR_NAME:
        _EXP_OPS = (dvo.__dict__["EXP_POLY4_ANT"], dvo.__dict__["EXP_SQ4_ANT"])
        return _EXP_OPS

    h = Src0 * C0 + C1
    h = h * Src0 + C2
    h = h * Src0 + One
    p = h * Src0 + One
    spec_poly = Spec(
        body=p,
        reference=lambda in0, in1, s0, s1, imm2: 1.0
        + in0 * (1.0 + in0 * (imm2 + in0 * (s1 + in0 * s0))),
    )
    spec_sq4 = Spec(
        body=sq(sq(sq(sq(Src0)))),
        reference=lambda in0, in1, s0, s1, imm2: (((in0**2) ** 2) ** 2) ** 2,
    )

    ops = []
    for name, spec in (("EXP_POLY4_ANT", spec_poly), ("EXP_SQ4_ANT", spec_sq4)):
        shas = {}
        for ver in ("v3", "v4"):
            uops = lower(spec, ver=ver)
            shas[ver] = DveOpSpec(
                name=name, opcode=0, uops=uops, rd1_en=False
            ).sha(ver)
        op = dvo.DveOp(name, spec, subdim=False, uops_sha=shas)
        dvo.OPS.append(op)
        dvo.CUSTOM_DVE_SPECS[name] = spec
        dvo._SUB_OPCODE_FOR_NAME[name] = dvo._CUSTOM_DVE_ROW_BASE + len(dvo.OPS) - 1
        ops.append(op)
    _EXP_OPS = tuple(ops)
    return _EXP_OPS


def _build(l=L):
    import concourse.bacc as bacc
    import concourse.tile as tile
    import concourse.mybir as mybir
    from concourse.masks import make_identity
    from contextlib import ExitStack

    exp_poly, exp_sq4 = _register_exp_ops()

    nt = l // TT
    ns = l // 128
    f32 = mybir.dt.float32
    f16 = mybir.dt.float16
    f32r = mybir.dt.float32r
    Exp = mybir.ActivationFunctionType.Exp
    add = mybir.AluOpType.add
    mult = mybir.AluOpType.mult

    nc = bacc.Bacc("TRN2", target_bir_lowering=False, debug=False,
                   num_devices=NCORES)

    x = nc.dram_tensor("x", [C, l], f16, kind="ExternalInput").ap()
    # wpack: [128, 12*128 (wq|wk|wv kt-tiles) + 512 (wp)] fp16
    wpack = nc.dram_tensor("wpack", [128, 2048], f16, kind="ExternalInput").ap()
    bq = nc.dram_tensor("bq", [128, 1], f32, kind="ExternalInput").ap()
    out = nc.dram_tensor("out", [C, l], f32, kind="ExternalOutput").ap()
    dbg = {}
    if DEBUG:
        dbg["z0"] = nc.dram_tensor("dbg_z0", [1, TT], f32,
                                   kind="ExternalOutput").ap()
        dbg["rz0"] = nc.dram_tensor("dbg_rz0", [1, TT], f32,
                                    kind="ExternalOutput").ap()
        dbg["zbs"] = nc.dram_tensor("dbg_zbs", [128, TT], f32,
                                    kind="ExternalOutput").ap()
        dbg["ou"] = nc.dram_tensor("dbg_ou", [128, TT], f32,
                                   kind="ExternalOutput").ap()
        dbg["osb"] = nc.dram_tensor("dbg_osb", [128, TT], f16,
                                    kind="ExternalOutput").ap()
        dbg["e0"] = nc.dram_tensor("dbg_e0", [128, 2 * TT], f32,
                                   kind="ExternalOutput").ap()

    DVW = 1024 - AW  # dve exp column count

    with tile.TileContext(nc) as tc, ExitStack() as ctx:
        persist = ctx.enter_context(tc.tile_pool(name="persist", bufs=1))
        e_pool = ctx.enter_context(tc.tile_pool(name="e", bufs=6))
        p_pool = ctx.enter_context(tc.tile_pool(name="p", bufs=2))
        o_pool = ctx.enter_context(tc.tile_pool(name="o", bufs=2))
        z_pool = ctx.enter_context(tc.tile_pool(name="z", bufs=2))
        res_pool = ctx.enter_context(tc.tile_pool(name="res", bufs=3))

        # ---- weights: one packed fp16 block, 4 chunked DMAs ----
        wall = persist.tile([128, 2048], f16, tag="wall")
        for c4 in range(4):
            nc.sync.dma_start(out=wall[:, c4 * 512:(c4 + 1) * 512],
                              in_=wpack[:, c4 * 512:(c4 + 1) * 512])
        w_r = wall[:, 0:1536].rearrange("p (i kt c) -> p i kt c", i=3, kt=KT)
        wp_r = wall[:, 1536:2048]

        bq_sb = persist.tile([128, 1], f32, tag="bq")
        nc.sync.dma_start(out=bq_sb, in_=bq)

        ident = persist.tile([128, 128], f16, tag="ident")
        make_identity(nc, ident)

        ones_r = persist.tile([1, 64], f32, tag="onesr")
        nc.vector.memset(ones_r, 1.0)

        # ---- persistent activations ----
        x_sb = persist.tile([128, KT, l], f16, tag="xsb")
        for n in range(l // TT):
            for kt in range(KT):
                nc.gpsimd.dma_start(
                    out=x_sb[:, kt, n * TT:(n + 1) * TT],
                    in_=x[kt * 128:(kt + 1) * 128, n * TT:(n + 1) * TT])
        q_sb = persist.tile([128, l], f16, tag="q")
        k_sb = persist.tile([128, l], f16, tag="k")
        vt_sb = persist.tile([128, ns, 130], f32r, tag="vt")
        # interleaved ones columns -> softmax denominator rows in AV psum
        ones_col = persist.tile([128, ns, 1], f32, tag="onescol")
        nc.vector.memset(ones_col, 1.0)
        nc.vector.tensor_copy(vt_sb[:, :, 64:65], ones_col)
        nc.vector.tensor_copy(vt_sb[:, :, 129:130], ones_col)

        # ================= QKV projections + V transpose =================
        with ExitStack() as qctx:
            qkv_ps = qctx.enter_context(
                tc.tile_pool(name="qkvps", bufs=2, space="PSUM"))
            v_pool = qctx.enter_context(tc.tile_pool(name="vsb", bufs=2))

            for n in range(nt):
                nsl = slice(n * TT, (n + 1) * TT)
                x_r = x_sb[:, :, nsl]

                q_ps = qkv_ps.tile([128, TT], f32, tag="qps")
                k_ps = qkv_ps.tile([128, TT], f32, tag="kps")
                v_ps = qkv_ps.tile([128, TT], f32, tag="vps")
                for wi, ps in enumerate((q_ps, k_ps, v_ps)):
                    for kt in range(KT):
                        nc.tensor.matmul(ps, w_r[:, wi, kt, :], x_r[:, kt, :],
                                         start=(kt == 0), stop=(kt == KT - 1))

                nc.vector.tensor_scalar(q_sb[:, nsl], q_ps,
                                        bq_sb, None, add)
                nc.scalar.copy(k_sb[:, nsl], k_ps)
                v_sb = v_pool.tile([128, TT], f16, tag="v")
                nc.vector.tensor_copy(v_sb, v_ps)

                # transpose V tile: 4 PE transposes -> [s, c] in psum
                tp = qkv_ps.tile([128, TT], f16, tag="tp")
                for j in range(4):
                    nc.tensor.transpose(tp[:, j * 128:(j + 1) * 128],
                                        v_sb[:, j * 128:(j + 1) * 128], ident)
                tp_v = tp.rearrange("p (j c) -> p j c", j=4)
                ssl = slice(4 * n, 4 * n + 4)
                nc.vector.tensor_copy(vt_sb[:, ssl, 0:64], tp_v[:, :, 0:64])
                nc.vector.tensor_copy(vt_sb[:, ssl, 65:129], tp_v[:, :, 64:128])

        # ========================= attention =========================
        with ExitStack() as actx:
            st_pool = actx.enter_context(
                tc.tile_pool(name="stps", bufs=2, space="PSUM"))
            av_pool = actx.enter_context(
                tc.tile_pool(name="avps", bufs=3, space="PSUM"))
            pr_pool = actx.enter_context(
                tc.tile_pool(name="prps", bufs=1, space="PSUM"))

            # Deferred tail work from the previous t-tile. The reciprocal +
            # denominator-broadcast chain is issued right at the t-boundary
            # (rcp first in the DVE queue, zbmm after two ST prefetches in
            # the PE queue); normalize muls and the partial projections are
            # spread over the next t-tile's s-loop at fixed slots, PE work
            # issued first within an iteration so stale deps never block
            # younger PE instructions in the in-order queue.
            def make_tail(av0, av1, tsl, dbg_t=False):
                rz0 = z_pool.tile([1, TT], f32, tag="rz0")
                rz1 = z_pool.tile([1, TT], f32, tag="rz1")
                zb = pr_pool.tile([128, TT], f32, tag="pp")
                zbs = z_pool.tile([128, TT], f32, tag="zbs")
                ou = z_pool.tile([128, TT], f32, tag="ou")
                o_sb = o_pool.tile([128, TT], f16, tag="o")

                zr0 = z_pool.tile([1, TT], f32, tag="zr0")
                zr1 = z_pool.tile([1, TT], f32, tag="zr1")

                def rcp():
                    # custom-DVE ops drop the partition offset of their in0,
                    # so stage the z rows to partition-0 tiles first
                    nc.vector.tensor_copy(zr0, av0[64:65, :])
                    nc.vector.reciprocal_approx_fast(out=rz0, in_=zr0)
                    nc.vector.tensor_copy(zr1, av1[64:65, :])
                    nc.vector.reciprocal_approx_fast(out=rz1, in_=zr1)
                    if dbg_t:
                        nc.sync.dma_start(out=dbg["z0"], in_=zr0)
                        nc.sync.dma_start(out=dbg["rz0"], in_=rz0)

                def zbmm():
                    nc.tensor.matmul(zb[0:64, :], ones_r,
                                     rz0, start=True, stop=True)
                    nc.tensor.matmul(zb[64:128, :], ones_r,
                                     rz1, start=True, stop=True)

                def ev0():
                    nc.vector.tensor_copy(ou[0:64, :], av0[0:64, :])

                def ev1():
                    nc.vector.tensor_copy(ou[64:128, :], av1[0:64, :])

                def zbcp():
                    nc.vector.tensor_copy(zbs, zb)

                def mul():
                    nc.vector.tensor_mul(o_sb, ou, zbs)
                    if dbg_t:
                        nc.sync.dma_start(out=dbg["zbs"], in_=zbs)
                        nc.sync.dma_start(out=dbg["ou"], in_=ou)
                        nc.sync.dma_start(out=dbg["osb"], in_=o_sb)

                work = [(0, False, ev0), (1, False, ev1), (2, True, zbmm),
                        (3, False, zbcp), (4, False, mul)]
                for i, ot in enumerate(range(KT)):
                    pp = pr_pool.tile([128, TT], f32, tag="pp")

                    def mmfn(pp=pp, ot=ot):
                        nc.tensor.matmul(pp, wp_r[:, ot * 128:(ot + 1) * 128],
                                         o_sb, start=True, stop=True)

                    def cpfn(pp=pp, ot=ot):
                        res = res_pool.tile([128, TT], f32, tag="res")
                        nc.vector.tensor_copy(res, pp)
                        nc.gpsimd.dma_start(
                            out=out[ot * 128:(ot + 1) * 128, tsl], in_=res)

                    work.append((8 + 4 * i, True, mmfn))
                    work.append((10 + 4 * i, False, cpfn))
                return rcp, work

            def issue_st_g(g):
                t2, s2 = divmod(g, ns)
                tsl2 = slice(t2 * TT, (t2 + 1) * TT)
                ssl = slice(s2 * 128, (s2 + 1) * 128)
                st_ps = st_pool.tile([128, 2 * TT], f32, tag="st")
                nc.tensor.matmul(st_ps[:, 0:TT], k_sb[0:64, ssl],
                                 q_sb[0:64, tsl2], start=True, stop=True)
                nc.tensor.matmul(st_ps[:, TT:2 * TT], k_sb[64:128, ssl],
                                 q_sb[64:128, tsl2], start=True, stop=True)
                return st_ps

            st_tiles = {}
            tail_rcp, tail = None, []
            for t in range(nt):
                tsl = slice(t * TT, (t + 1) * TT)
                av0 = av_pool.tile([128, TT], f32, tag="av")
                av1 = av_pool.tile([128, TT], f32, tag="av")

                h1q, e_tiles = [], {}
                if t == 0:
                    st_tiles[0] = issue_st_g(0)
                    st_tiles[1] = issue_st_g(1)
                if tail_rcp is not None:
                    tail_rcp()

                for s in range(ns):
                    for slot, is_pe, fn in tail:
                        if slot == s and is_pe:
                            with tc.high_priority(offset=-700):
                                fn()
                    g = t * ns + s + 2
                    if g < nt * ns:
                        st_tiles[g] = issue_st_g(g)
                    st_ps = st_tiles.pop(t * ns + s)

                    e_sb = e_pool.tile([128, 2 * TT], f32r, tag="e")
                    nc.scalar.activation(e_sb[:, 0:AW], st_ps[:, 0:AW], Exp,
                                         scale=16.0)
                    if DVW:
                        p_sb = p_pool.tile([128, DVW], f32, tag="p")
                        nc.vector._custom_dve(exp_poly, out=p_sb,
                                              in0=st_ps[:, AW:2 * TT],
                                              s0=1.0 / 24.0, s1=1.0 / 6.0,
                                              imm2=0.5)
                        nc.vector._custom_dve(exp_sq4,
                                              out=e_sb[:, AW:2 * TT], in0=p_sb)

                    if DEBUG and t == 0 and s == 0:
                        nc.sync.dma_start(out=dbg["e0"],
                                          in_=e_sb.bitcast(f32))
                    nc.tensor.matmul(av0[0:65, :], vt_sb[:, s, 0:65],
                                     e_sb[:, 0:TT], start=(s == 0),
                                     stop=(s == ns - 1))
                    # h1 AV stream runs ~4 s-tiles behind h0 (PSUM accumulate
                    # order is free) so the first h1 write of a t-tile lands
                    # after the previous tile's av slot is drained
                    e_tiles[s] = e_sb
                    h1q.append(s)
                    npop = 2 if 4 <= s < 8 else (1 if s >= 8 else 0)
                    for _ in range(npop):
                        j = h1q.pop(0)
                        ej = e_tiles.pop(j)
                        nc.tensor.matmul(av1[0:65, :], vt_sb[:, j, 65:130],
                                         ej[:, TT:2 * TT], start=(j == 0),
                                         stop=(j == ns - 1))

                    for slot, is_pe, fn in tail:
                        if slot == s and not is_pe:
                            with tc.high_priority(offset=-700):
                                fn()
                tail_rcp, tail = make_tail(av0, av1, tsl,
                                           dbg_t=(DEBUG and t == 0))

            # epilogue: flush the final t-tile's tail work
            tail_rcp()
            for slot, is_pe, fn in sorted(tail, key=lambda w: w[0]):
                fn()

    nc.compile()
    return nc


def _get_nc(l=L):
    if l not in _BUILT:
        _BUILT[l] = _build(l)
    return _BUILT[l]


def _shard_inputs(x, Wq, bq, Wkv, bkv, Wp, bp, l=L):
    x = np.asarray(x, dtype=np.float32)
    Wq = np.asarray(Wq, dtype=np.float32)
    bq = np.asarray(bq, dtype=np.float32)
    Wkv = np.asarray(Wkv, dtype=np.float32)
    bkv = np.asarray(bkv, dtype=np.float32)
    Wp = np.asarray(Wp, dtype=np.float32)
    bp = np.asarray(bp, dtype=np.float32)

    s16 = SCALE / 16.0
    in_maps = []
    for core in range(NCORES):
        b, hp = divmod(core, 4)
        sl = slice(hp * 128, (hp + 1) * 128)
        vsl = slice(C + hp * 128, C + (hp + 1) * 128)
        wq_t = (Wq[sl, :] * s16).T.astype(np.float16)      # [C, 128]
        wk_t = Wkv[sl, :].T.astype(np.float16)
        wv_t = Wkv[vsl, :].T.astype(np.float16)
        wp_t = Wp[:, sl].T.astype(np.float16)               # [128, C]
        wpack = np.empty((128, 2048), dtype=np.float16)
        for i, w in enumerate((wq_t, wk_t, wv_t)):
            for kt in range(KT):
                wpack[:, (i * KT + kt) * 128:(i * KT + kt + 1) * 128] = \
                    w[kt * 128:(kt + 1) * 128, :]
        wpack[:, 1536:2048] = wp_t
        m = {
            "x": np.ascontiguousarray(x[b].reshape(C, l).astype(np.float16)),
            "wpack": np.ascontiguousarray(wpack),
            "bq": np.ascontiguousarray(
                (bq[sl] * s16).reshape(128, 1).astype(np.float32)),
        }
        in_maps.append(m)
    return in_maps


def _host_const(Wkv, bkv, Wp, bp):
    """Bias terms folded out of the kernel: out += Wp @ bv + bp (exact)."""
    Wkv = np.asarray(Wkv, dtype=np.float64)
    bkv = np.asarray(bkv, dtype=np.float64)
    Wp = np.asarray(Wp, dtype=np.float64)
    bp = np.asarray(bp, dtype=np.float64)
    bv = bkv[C:]
    return (Wp @ bv + bp).astype(np.float32)


def _run(in_maps, l=L, trace=False):
    from concourse.bass_utils import run_bass_kernel_spmd
    nc = _get_nc(l)
    return run_bass_kernel_spmd(nc, in_maps, core_ids=list(range(NCORES)),
                                trace=trace)


def kernel(x, Wq, bq, Wkv, bkv, Wp, bp):
    in_maps = _shard_inputs(x, Wq, bq, Wkv, bkv, Wp, bp)
    res = _run(in_maps)
    outs = [res.results[i]["out"] for i in range(NCORES)]
    cv = _host_const(Wkv, bkv, Wp, bp)[None, :, None]
    y = np.stack([outs[0] + outs[1] + outs[2] + outs[3],
                  outs[4] + outs[5] + outs[6] + outs[7]]) + cv
    return np.ascontiguousarray(y.reshape(B, C, HH, WW), dtype=np.float32)
